# revision 16
# baseline (speedup 1.0000x reference)
"""Ragged segment mean kernel for Trainium2 (8 NeuronCores, data-parallel).

Problem: seq [64, 2048, 1024] f32, begin/end [64] i64.
Output: out[i] = mean(seq[i, begin[i]:end[i], :])  -> [64, 1024] f32.

Strategy: data parallel over segment ROWS. The host packs the 64
segments (seq[i, begin:end]) back to back in a bin-packed order and
cuts the packed row list into 8 equal contiguous shards of Q rows, one
per core (a segment may straddle a shard boundary; its partial means
are summed on the host, which is exact because the mask carries
1/count). Each core's input is its own packed shard padded with zeros
to a common R_cap rows, so the device reads exactly the segment bytes
at statically known offsets with perfect row-granularity balance: no
index DMAs, no registers, no bounds checks.

Per 128-row chunk the PE computes acc[16, 512] += m[128, 16].T @
chunk[128, 512] accumulated in PSUM over all chunks. The host-built
mask m carries 1/count in the rows belonging to output column c and 0
elsewhere (zero padding included), so PSUM directly accumulates the
segment MEAN and no separate scale pass is needed.

fp32 matmuls stream at 4 cycles/row on the PE, which would bottleneck.
The packed rows are typed float32r end-to-end instead (same 32-bit
container, 1 cycle/row for free dim >= 256); the PE rounds f32r
operands internally (~1e-4 relative), well inside the 2e-2 gate. The
DMA stream is then the only bottleneck; the measured SDMA engines run
~98% busy for the whole kernel body.

Raw bass (no TileContext): the dependence structure is a plain linear
pipeline, so hand-rolled semaphores avoid the Tile prologue/teardown
barriers. Slot DMAs are issued from both HWDGE rings (SP + ACT) to
shorten the issue ramp, and the drain is split by PSUM bank: the
second 512-column half of the result is still accumulating while the
first half is already being copied out and stored.

The slot schedule is [512-row x N, then 256/128 tapers] summing to
R_cap; compiled kernels are cached per R_cap (input-dependent), so
unusual inputs at worst trigger a recompile, never a wrong result.
"""

import contextlib

import numpy as np

import concourse.bacc as bacc
import concourse.bass as bass
import concourse.mybir as mybir
from concourse.bass_utils import run_bass_kernel_spmd

B, L, D = 64, 2048, 1024
NCORES = 8
BP = B // NCORES              # 8 samples per core in the bin-packing
M = 16                        # output columns per core (straddle slack)
FREE = 512                    # PSUM bank limit for matmul N
NMM = D // FREE               # 2 matmuls per 128-row chunk
TGF = 512 * D // 128          # tile free size (512-row slot)

_nc_cache = {}


def _schedule(r_cap):
    """Slot sizes summing to r_cap.

    A 128-row slot leads (short first descriptor-generation pass, so
    the SDMA engines start streaming sooner), 512s carry the bulk, and
    a 256/128 taper trails (short end-of-stream drain chain).
    """
    sizes = []
    rem = r_cap
    if rem > 1536:
        sizes.append(128)
        rem -= 128
    while rem > 1024:
        sizes.append(512)
        rem -= 512
    while rem > 256:
        sizes.append(256)
        rem -= 256
    while rem > 0:
        sizes.append(128)
        rem -= 128
    return sizes


def _build_nc(r_cap):
    POS = _schedule(r_cap)
    NPOS = len(POS)
    NCH = r_cap // 128
    NB = min(6, NPOS)  # in-flight slot buffers
    nc = bacc.Bacc("TRN2", target_bir_lowering=False)
    f32 = mybir.dt.float32
    f32r = mybir.dt.float32r
    seq = nc.dram_tensor("seq", [r_cap, D], f32r, kind="ExternalInput")
    maskt = nc.dram_tensor("maskt", [128, NCH * M], f32r, kind="ExternalInput")
    out = nc.dram_tensor("out", [M, D], f32, kind="ExternalOutput")

    # slots[i] = (row offset, rows, tile free size, chunk base)
    slots = []
    off = 0
    ch = 0
    for rows in POS:
        slots.append((off, rows, rows * D // 128, ch))
        off += rows
        ch += rows // 128

    with contextlib.ExitStack() as ctx:
        buf = ctx.enter_context(nc.sbuf_tensor("bufs", [128, NB * TGF], f32r))
        mr = ctx.enter_context(nc.sbuf_tensor("mr", [128, NCH * M], f32r))
        res = ctx.enter_context(nc.sbuf_tensor("res", [M, D], f32))
        acc = ctx.enter_context(nc.psum_tensor("acc", [M, D], f32))
        warm = ctx.enter_context(nc.psum_tensor("warm", [M, FREE], f32))
        bsems = [
            ctx.enter_context(nc.semaphore(f"bsem{k}")) for k in range(NB)
        ]
        msem = ctx.enter_context(nc.semaphore("msem"))
        psem = ctx.enter_context(nc.semaphore("psem"))
        c0sem = ctx.enter_context(nc.semaphore("c0sem"))
        vsem = ctx.enter_context(nc.semaphore("vsem"))
        osem = ctx.enter_context(nc.semaphore("osem"))
        sem_nums = [
            s.num for s in bsems + [msem, psem, c0sem, vsem, osem]
        ]

        def slot_dma(eng, i):
            off, rows, gf, ch0 = slots[i]
            k = i % NB
            src = seq[off : off + rows, :].rearrange("(p j) d -> p (j d)", p=128)
            eng.dma_start(out=buf[:, k * TGF : k * TGF + gf], in_=src).then_inc(
                bsems[k], 16
            )

        with nc.Block(no_gpsimd_drain=True):

            def sp_prog(sync):
                for i in range(NPOS):
                    if i >= NB:
                        # buffer free once the PE retired slot i-NB
                        sync.wait_ge(psem, i - NB + 1)
                    slot_dma(sync, i)
                # bank-split store: first half as soon as DVE copied it
                # (the second half goes out on the ACT ring in parallel)
                sync.wait_ge(vsem, 1)
                sync.dma_start(out=out[:, 0:FREE], in_=res[:, 0:FREE]).then_inc(
                    osem, 16
                )
                # program end = output landed in HBM
                sync.wait_ge(osem, 32)

            def act_prog(scalar):
                # mask DMA rides the ACT HWDGE ring, concurrent with the
                # slot stream starting on the SP ring
                scalar.dma_start(out=mr[:], in_=maskt[:]).then_inc(msem, 16)
                scalar.wait_ge(vsem, 2)
                scalar.dma_start(out=out[:, FREE:D], in_=res[:, FREE:D]).then_inc(
                    osem, 16
                )

            def pe_prog(tensor):
                tensor.wait_ge(msem, 16)
                # warmup matmul consuming only the mask tile so real
                # matmuls' waits cover only the seq pipeline
                nc.tensor.matmul(
                    out=warm[:, 0:M],
                    lhsT=mr[:, 0:M],
                    rhs=mr[:, 0:M],
                    start=True,
                    stop=True,
                )
                for i in range(NPOS):
                    off, rows, gf, ch0 = slots[i]
                    k = i % NB
                    jpg = rows // 128
                    last = i == NPOS - 1
                    # keep-warm fillers: run while the sequencer would
                    # otherwise idle at the wait below, so the HAM clock
                    # gate never re-throttles the PE to 1.2 GHz between
                    # slots (cold matmuls cost 2x and stretch the drain)
                    for _ in range(4):
                        nc.tensor.matmul(
                            out=warm[:],
                            lhsT=mr[:, 0:M],
                            rhs=mr[:, 0:FREE],
                            start=True,
                            stop=True,
                        )
                    tensor.wait_ge(bsems[k], 16 * (i // NB + 1))
                    for h in range(NMM):
                        # h-major on the final slot: bank 0 finishes (and
                        # retires via c0sem) before bank 1's matmuls run
                        for j in range(jpg):
                            lhs = mr[:, (ch0 + j) * M : (ch0 + j + 1) * M]
                            base = k * TGF + j * D + h * FREE
                            mm = nc.tensor.matmul(
                                out=acc[:, h * FREE : (h + 1) * FREE],
                                lhsT=lhs,
                                rhs=buf[:, base : base + FREE],
                                start=(i == 0 and j == 0),
                                stop=(last and j == jpg - 1),
                            )
                        if last and h == 0:
                            mm.then_inc(c0sem, 1)
                    # retire marker: all reads of buffer k for slot i done
                    mm.then_inc(psem, 1)

            def dve_prog(vector):
                vector.wait_ge(c0sem, 1)
                nc.vector.tensor_copy(
                    out=res[:, 0:FREE], in_=acc[:, 0:FREE]
                ).then_inc(vsem, 1)
                vector.wait_ge(psem, NPOS)
                nc.vector.tensor_copy(
                    out=res[:, FREE:D], in_=acc[:, FREE:D]
                ).then_inc(vsem, 1)

            def gp_prog(gpsimd):
                # re-zero kernel semaphores so a re-execution of this
                # loaded NEFF starts from a clean state
                gpsimd.wait_ge(osem, 32)
                for rng in bass.compact_to_ranges(sem_nums):
                    gpsimd.dma_reset(rng)
                    gpsimd.sem_clear(rng)

            blk = nc.cur_block
            blk.sync(sp_prog)
            blk.scalar(act_prog)
            blk.tensor(pe_prog)
            blk.vector(dve_prog)
            blk.gpsimd(gp_prog)
    nc.compile()
    return nc


def _plan(begin, end):
    """Pack segments contiguously, cut into 8 equal row shards.

    Returns (orderd, cuts, q, r_cap):
      orderd -- sample indices in packed order
      cuts   -- per-core global row ranges [(a, b)] with b-a <= r_cap
    """
    span = (end - begin).astype(np.int64)
    total = int(span.sum())
    # bin-pack samples (8 per core) so an equal-row cut of the packed
    # order straddles few segments per core
    order = np.argsort(-span, kind="stable")
    loads = [0] * NCORES
    members = [[] for _ in range(NCORES)]
    for si in order:
        avail = [c for c in range(NCORES) if len(members[c]) < BP]
        ci = min(avail, key=lambda c: loads[c])
        loads[ci] += int(span[si])
        members[ci].append(int(si))
    orderd = [si for ci in range(NCORES) for si in members[ci]]
    q = -(-total // NCORES)
    cuts = [(c * q, min((c + 1) * q, total)) for c in range(NCORES)]
    # per-core distinct samples must fit the M output columns; if not
    # (pathological inputs), fall back to sample-aligned cuts
    starts = np.cumsum([0] + [int(span[si]) for si in orderd])
    def _ncols(a, b):
        i0 = int(np.searchsorted(starts, a, "right")) - 1
        i1 = int(np.searchsorted(starts, b, "left"))
        return i1 - i0
    if any(_ncols(a, b) > M for a, b in cuts):
        bounds = np.cumsum([0] + loads)
        cuts = [(int(bounds[c]), int(bounds[c + 1])) for c in range(NCORES)]
    rows_max = max(b - a for a, b in cuts)
    r_cap = -(-max(rows_max, 128) // 128) * 128
    return orderd, cuts, r_cap


def _make_in_maps(seq, begin, end, orderd, cuts, r_cap):
    POS = _schedule(r_cap)
    NCH = r_cap // 128
    p = np.arange(128)
    span = (end - begin).astype(np.int64)
    starts = np.cumsum([0] + [int(span[si]) for si in orderd])
    in_maps = []
    colmaps = []  # per core: list of (sample, col)
    for a, b in cuts:
        packed = np.zeros((r_cap, D), dtype=np.float32)
        w = np.zeros(r_cap, dtype=np.float64)
        col = np.full(r_cap, -1, dtype=np.int64)
        cmap = []
        i0 = int(np.searchsorted(starts, a, "right")) - 1
        r0 = 0
        for idx in range(i0, len(orderd)):
            if starts[idx] >= b:
                break
            si = orderd[idx]
            lo = max(a, int(starts[idx]))
            hi = min(b, int(starts[idx + 1]))
            if hi <= lo:
                continue
            sb = int(begin[si]) + (lo - int(starts[idx]))
            n = hi - lo
            packed[r0 : r0 + n] = seq[si, sb : sb + n]
            c = len(cmap)
            cmap.append((int(si), c))
            col[r0 : r0 + n] = c
            w[r0 : r0 + n] = 1.0 / int(span[si])
            r0 += n
        assert len(cmap) <= M
        mt = np.zeros((128, NCH * M), dtype=np.float32)
        off = 0
        ch = 0
        for rows in POS:
            jpg = rows // 128
            for j in range(jpg):
                # slot tile[p, j*D+d] holds packed row off + p*jpg + j
                r = off + p * jpg + j
                base = (ch + j) * M
                for i in range(len(cmap)):
                    mt[:, base + i] = np.where(col[r] == i, w[r], 0.0).astype(
                        np.float32
                    )
            ch += jpg
            off += rows
        in_maps.append({"seq": packed, "maskt": mt})
        colmaps.append(cmap)
    return in_maps, colmaps


def _axon_reset():
    """Best-effort NeuronCore reset (recovers a device wedged by an
    earlier failed run in the same container)."""
    try:
        import ctypes

        import jax

        jax.devices()
        lib = ctypes.CDLL("/opt/axon/libaxon_pjrt.so")
        lib.axon_reset.restype = ctypes.c_int64
        lib.axon_reset()
    except Exception:
        pass


def _run(seq, begin, end, trace=False):
    seq = np.asarray(seq)
    begin = np.asarray(begin).astype(np.int64)
    end = np.asarray(end).astype(np.int64)
    orderd, cuts, r_cap = _plan(begin, end)
    if r_cap not in _nc_cache:
        _nc_cache[r_cap] = _build_nc(r_cap)
    in_maps, colmaps = _make_in_maps(seq, begin, end, orderd, cuts, r_cap)
    try:
        res = run_bass_kernel_spmd(
            _nc_cache[r_cap], in_maps, list(range(NCORES)), trace=trace
        )
    except Exception:
        _axon_reset()
        res = run_bass_kernel_spmd(
            _nc_cache[r_cap], in_maps, list(range(NCORES)), trace=trace
        )
    out = np.zeros((B, D), dtype=np.float32)
    for ci in range(NCORES):
        part = res.results[ci]["out"]
        for si, c in colmaps[ci]:
            out[si] += part[c]
    return out, res


def kernel(seq, begin, end):
    out, _ = _run(seq, begin, end, trace=False)
    return out


# revision 17
# speedup vs baseline: 1.0063x; 1.0063x over previous
"""Ragged segment mean kernel for Trainium2 (8 NeuronCores, data-parallel).

Problem: seq [64, 2048, 1024] f32, begin/end [64] i64.
Output: out[i] = mean(seq[i, begin[i]:end[i], :])  -> [64, 1024] f32.

Strategy: data parallel over segment ROWS. The host packs the 64
segments (seq[i, begin:end]) back to back in a bin-packed order and
cuts the packed row list into 8 equal contiguous shards of Q rows, one
per core (a segment may straddle a shard boundary; its partial means
are summed on the host, which is exact because the mask carries
1/count). Each core's input is its own packed shard padded with zeros
to a common R_cap rows, so the device reads exactly the segment bytes
at statically known offsets with perfect row-granularity balance: no
index DMAs, no registers, no bounds checks.

Per 128-row chunk the PE computes acc[16, 512] += m[128, 16].T @
chunk[128, 512] accumulated in PSUM over all chunks. The host-built
mask m carries 1/count in the rows belonging to output column c and 0
elsewhere (zero padding included), so PSUM directly accumulates the
segment MEAN and no separate scale pass is needed.

fp32 matmuls stream at 4 cycles/row on the PE, which would bottleneck.
The packed rows are typed float32r end-to-end instead (same 32-bit
container, 1 cycle/row for free dim >= 256); the PE rounds f32r
operands internally (~1e-4 relative), well inside the 2e-2 gate. The
DMA stream is then the only bottleneck; the measured SDMA engines run
~98% busy for the whole kernel body.

Raw bass (no TileContext): the dependence structure is a plain linear
pipeline, so hand-rolled semaphores avoid the Tile prologue/teardown
barriers. Slot DMAs are issued from both HWDGE rings (SP + ACT) to
shorten the issue ramp, and the drain is split by PSUM bank: the
second 512-column half of the result is still accumulating while the
first half is already being copied out and stored.

The slot schedule is [512-row x N, then 256/128 tapers] summing to
R_cap; compiled kernels are cached per R_cap (input-dependent), so
unusual inputs at worst trigger a recompile, never a wrong result.
"""

import contextlib

import numpy as np

import concourse.bacc as bacc
import concourse.bass as bass
import concourse.mybir as mybir
from concourse.bass_utils import run_bass_kernel_spmd

B, L, D = 64, 2048, 1024
NCORES = 8
BP = B // NCORES              # 8 samples per core in the bin-packing
M = 16                        # output columns per core (straddle slack)
FREE = 512                    # PSUM bank limit for matmul N
NMM = D // FREE               # 2 matmuls per 128-row chunk
TGF = 512 * D // 128          # tile free size (512-row slot)

_nc_cache = {}


def _schedule(r_cap):
    """Slot sizes summing to r_cap.

    A 128-row slot leads (short first descriptor-generation pass, so
    the SDMA engines start streaming sooner), 512s carry the bulk, and
    a 256/128 taper trails (short end-of-stream drain chain).
    """
    sizes = []
    rem = r_cap
    if rem > 1536:
        sizes.append(128)
        rem -= 128
    while rem > 1024:
        sizes.append(512)
        rem -= 512
    while rem > 256:
        sizes.append(256)
        rem -= 256
    while rem > 0:
        sizes.append(128)
        rem -= 128
    return sizes


def _build_nc(r_cap):
    POS = _schedule(r_cap)
    NPOS = len(POS)
    NCH = r_cap // 128
    NB = min(6, NPOS)  # in-flight slot buffers
    nc = bacc.Bacc("TRN2", target_bir_lowering=False)
    f32 = mybir.dt.float32
    f32r = mybir.dt.float32r
    seq = nc.dram_tensor("seq", [r_cap, D], f32r, kind="ExternalInput")
    maskt = nc.dram_tensor("maskt", [128, NCH * M], f32r, kind="ExternalInput")
    out = nc.dram_tensor("out", [M, D], f32, kind="ExternalOutput")

    # slots[i] = (row offset, rows, tile free size, chunk base)
    slots = []
    off = 0
    ch = 0
    for rows in POS:
        slots.append((off, rows, rows * D // 128, ch))
        off += rows
        ch += rows // 128

    with contextlib.ExitStack() as ctx:
        buf = ctx.enter_context(nc.sbuf_tensor("bufs", [128, NB * TGF], f32r))
        mr = ctx.enter_context(nc.sbuf_tensor("mr", [128, NCH * M], f32r))
        res = ctx.enter_context(nc.sbuf_tensor("res", [M, D], f32))
        acc = ctx.enter_context(nc.psum_tensor("acc", [M, D], f32))
        warm = ctx.enter_context(nc.psum_tensor("warm", [M, FREE], f32))
        bsems = [
            ctx.enter_context(nc.semaphore(f"bsem{k}")) for k in range(NB)
        ]
        msem = ctx.enter_context(nc.semaphore("msem"))
        psem = ctx.enter_context(nc.semaphore("psem"))
        c0sem = ctx.enter_context(nc.semaphore("c0sem"))
        vsem = ctx.enter_context(nc.semaphore("vsem"))
        osem = ctx.enter_context(nc.semaphore("osem"))
        sem_nums = [
            s.num for s in bsems + [msem, psem, c0sem, vsem, osem]
        ]

        def slot_dma(eng, i):
            off, rows, gf, ch0 = slots[i]
            k = i % NB
            src = seq[off : off + rows, :].rearrange("(p j) d -> p (j d)", p=128)
            eng.dma_start(out=buf[:, k * TGF : k * TGF + gf], in_=src).then_inc(
                bsems[k], 16
            )

        with nc.Block(no_gpsimd_drain=True):

            def sp_prog(sync):
                for i in range(NPOS):
                    if i >= NB:
                        # buffer free once the PE retired slot i-NB
                        sync.wait_ge(psem, i - NB + 1)
                    slot_dma(sync, i)
                # bank-split store: first half as soon as DVE copied it
                # (the second half goes out on the ACT ring in parallel)
                sync.wait_ge(vsem, 1)
                sync.dma_start(out=out[:, 0:FREE], in_=res[:, 0:FREE]).then_inc(
                    osem, 16
                )
                # program end = output landed in HBM
                sync.wait_ge(osem, 32)

            def act_prog(scalar):
                # mask DMA rides the ACT HWDGE ring, concurrent with the
                # slot stream starting on the SP ring
                scalar.dma_start(out=mr[:], in_=maskt[:]).then_inc(msem, 16)
                scalar.wait_ge(vsem, 2)
                scalar.dma_start(out=out[:, FREE:D], in_=res[:, FREE:D]).then_inc(
                    osem, 16
                )

            def pe_prog(tensor):
                tensor.wait_ge(msem, 16)
                # warmup matmul consuming only the mask tile so real
                # matmuls' waits cover only the seq pipeline
                nc.tensor.matmul(
                    out=warm[:, 0:M],
                    lhsT=mr[:, 0:M],
                    rhs=mr[:, 0:M],
                    start=True,
                    stop=True,
                )
                for i in range(NPOS):
                    off, rows, gf, ch0 = slots[i]
                    k = i % NB
                    jpg = rows // 128
                    last = i == NPOS - 1
                    tensor.wait_ge(bsems[k], 16 * (i // NB + 1))
                    for h in range(NMM):
                        # h-major on the final slot: bank 0 finishes (and
                        # retires via c0sem) before bank 1's matmuls run
                        for j in range(jpg):
                            lhs = mr[:, (ch0 + j) * M : (ch0 + j + 1) * M]
                            base = k * TGF + j * D + h * FREE
                            mm = nc.tensor.matmul(
                                out=acc[:, h * FREE : (h + 1) * FREE],
                                lhsT=lhs,
                                rhs=buf[:, base : base + FREE],
                                start=(i == 0 and j == 0),
                                stop=(last and j == jpg - 1),
                            )
                        if last and h == 0:
                            mm.then_inc(c0sem, 1)
                    # retire marker: all reads of buffer k for slot i done
                    mm.then_inc(psem, 1)

            def dve_prog(vector):
                vector.wait_ge(c0sem, 1)
                nc.vector.tensor_copy(
                    out=res[:, 0:FREE], in_=acc[:, 0:FREE]
                ).then_inc(vsem, 1)
                vector.wait_ge(psem, NPOS)
                nc.vector.tensor_copy(
                    out=res[:, FREE:D], in_=acc[:, FREE:D]
                ).then_inc(vsem, 1)

            def gp_prog(gpsimd):
                # re-zero kernel semaphores so a re-execution of this
                # loaded NEFF starts from a clean state
                gpsimd.wait_ge(osem, 32)
                for rng in bass.compact_to_ranges(sem_nums):
                    gpsimd.dma_reset(rng)
                    gpsimd.sem_clear(rng)

            blk = nc.cur_block
            blk.sync(sp_prog)
            blk.scalar(act_prog)
            blk.tensor(pe_prog)
            blk.vector(dve_prog)
            blk.gpsimd(gp_prog)
    nc.compile()
    return nc


def _plan(begin, end):
    """Pack segments contiguously, cut into 8 equal row shards.

    Returns (orderd, cuts, q, r_cap):
      orderd -- sample indices in packed order
      cuts   -- per-core global row ranges [(a, b)] with b-a <= r_cap
    """
    span = (end - begin).astype(np.int64)
    total = int(span.sum())
    # bin-pack samples (8 per core) so an equal-row cut of the packed
    # order straddles few segments per core
    order = np.argsort(-span, kind="stable")
    loads = [0] * NCORES
    members = [[] for _ in range(NCORES)]
    for si in order:
        avail = [c for c in range(NCORES) if len(members[c]) < BP]
        ci = min(avail, key=lambda c: loads[c])
        loads[ci] += int(span[si])
        members[ci].append(int(si))
    orderd = [si for ci in range(NCORES) for si in members[ci]]
    q = -(-total // NCORES)
    cuts = [(c * q, min((c + 1) * q, total)) for c in range(NCORES)]
    # per-core distinct samples must fit the M output columns; if not
    # (pathological inputs), fall back to sample-aligned cuts
    starts = np.cumsum([0] + [int(span[si]) for si in orderd])
    def _ncols(a, b):
        i0 = int(np.searchsorted(starts, a, "right")) - 1
        i1 = int(np.searchsorted(starts, b, "left"))
        return i1 - i0
    if any(_ncols(a, b) > M for a, b in cuts):
        bounds = np.cumsum([0] + loads)
        cuts = [(int(bounds[c]), int(bounds[c + 1])) for c in range(NCORES)]
    rows_max = max(b - a for a, b in cuts)
    r_cap = -(-max(rows_max, 128) // 128) * 128
    return orderd, cuts, r_cap


def _make_in_maps(seq, begin, end, orderd, cuts, r_cap):
    POS = _schedule(r_cap)
    NCH = r_cap // 128
    p = np.arange(128)
    span = (end - begin).astype(np.int64)
    starts = np.cumsum([0] + [int(span[si]) for si in orderd])
    in_maps = []
    colmaps = []  # per core: list of (sample, col)
    for a, b in cuts:
        packed = np.zeros((r_cap, D), dtype=np.float32)
        w = np.zeros(r_cap, dtype=np.float64)
        col = np.full(r_cap, -1, dtype=np.int64)
        cmap = []
        i0 = int(np.searchsorted(starts, a, "right")) - 1
        r0 = 0
        for idx in range(i0, len(orderd)):
            if starts[idx] >= b:
                break
            si = orderd[idx]
            lo = max(a, int(starts[idx]))
            hi = min(b, int(starts[idx + 1]))
            if hi <= lo:
                continue
            sb = int(begin[si]) + (lo - int(starts[idx]))
            n = hi - lo
            packed[r0 : r0 + n] = seq[si, sb : sb + n]
            c = len(cmap)
            cmap.append((int(si), c))
            col[r0 : r0 + n] = c
            w[r0 : r0 + n] = 1.0 / int(span[si])
            r0 += n
        assert len(cmap) <= M
        mt = np.zeros((128, NCH * M), dtype=np.float32)
        off = 0
        ch = 0
        for rows in POS:
            jpg = rows // 128
            for j in range(jpg):
                # slot tile[p, j*D+d] holds packed row off + p*jpg + j
                r = off + p * jpg + j
                base = (ch + j) * M
                for i in range(len(cmap)):
                    mt[:, base + i] = np.where(col[r] == i, w[r], 0.0).astype(
                        np.float32
                    )
            ch += jpg
            off += rows
        in_maps.append({"seq": packed, "maskt": mt})
        colmaps.append(cmap)
    return in_maps, colmaps


def _axon_reset():
    """Best-effort NeuronCore reset (recovers a device wedged by an
    earlier failed run in the same container)."""
    try:
        import ctypes

        import jax

        jax.devices()
        lib = ctypes.CDLL("/opt/axon/libaxon_pjrt.so")
        lib.axon_reset.restype = ctypes.c_int64
        lib.axon_reset()
    except Exception:
        pass


def _run(seq, begin, end, trace=False):
    seq = np.asarray(seq)
    begin = np.asarray(begin).astype(np.int64)
    end = np.asarray(end).astype(np.int64)
    orderd, cuts, r_cap = _plan(begin, end)
    if r_cap not in _nc_cache:
        _nc_cache[r_cap] = _build_nc(r_cap)
    in_maps, colmaps = _make_in_maps(seq, begin, end, orderd, cuts, r_cap)
    try:
        res = run_bass_kernel_spmd(
            _nc_cache[r_cap], in_maps, list(range(NCORES)), trace=trace
        )
    except Exception:
        _axon_reset()
        res = run_bass_kernel_spmd(
            _nc_cache[r_cap], in_maps, list(range(NCORES)), trace=trace
        )
    out = np.zeros((B, D), dtype=np.float32)
    for ci in range(NCORES):
        part = res.results[ci]["out"]
        for si, c in colmaps[ci]:
            out[si] += part[c]
    return out, res


def kernel(seq, begin, end):
    out, _ = _run(seq, begin, end, trace=False)
    return out
